# revision 1
# baseline (speedup 1.0000x reference)
"""Bass/Trainium2 kernel for nn_DisentangleLayer (FactorGCN-style GNN layer).

Math (per reference):
  h    = x @ W_lin + b_lin                    [N, 256]
  h_em = x @ emb_w + emb_b                    [N, 64]
  s_src = h @ a_src.T ; s_dst = h @ a_dst.T   [N, 4]    (att_w = [a_src | a_dst])
  e    = sigmoid(s_src[src] + s_dst[dst] + att_b)       [E, 4]
  ev   = exp(e - max(e))     (max subtraction cancels in the normalized
                              ratio below; sigmoid output is bounded so no
                              overflow risk -> we skip it)
  denom = segsum_src(ev)                       [N, 4]
  out[n, 64l:64l+64] = segsum_src(ev_l * h_em[dst]) / denom[n, l]

Strategy:
  * Host shards edges by src-range across 8 cores (each core owns 12500
    nodes' outputs; no cross-core reduction needed).
  * Per core, edges are sorted by src and mapped to dense ranks; ranks are
    grouped into 128-node windows; each window's edge list is padded to a
    fixed number of 128-edge tiles (uniform across cores -> one compiled
    program).
  * Device: phase A computes a packed per-node table
    [h_em(64) | s_dst+att_b(4) | s_src(4) | pad] (f32, 512B rows) for all
    N nodes (replicated per core).  Edge phase gathers table rows by dst
    via indirect DMA, forms per-tile one-hot matrices S (edges x ranks)
    and S^T on-chip, and uses TensorE matmuls for the per-window segment
    sums of [ev*h_em | ev]; normalization by denom happens per window.
"""

import math
import numpy as np
from contextlib import ExitStack

P = 128
CORES = 8
IN_F = 256
D_EM = 64
L = 4

_PATCHED = False


def _apply_tile_patch():
    """walrus in this env rejects >1 sem-wait on one instruction; split the
    TileContext exit-drain waits across single-wait nops."""
    global _PATCHED
    if _PATCHED:
        return
    _PATCHED = True
    import concourse.tile as tile_mod
    import concourse.mybir as mybir
    from concourse.vector_clock import ScopedClock

    def _drain_and_barrier(self, tick_clock, wait_clock):
        nop = self.nc.sync.nop()
        wait_clock.add_sem_waits(nop.ins, ScopedClock({None: tick_clock.global_clock}))
        si = nop.ins.sync_info
        waits = list(si.on_wait) if si is not None else []
        if len(waits) > 1:
            si.on_wait = waits[:1]
            nop.ins.sync_info = si
            for i in range(1, len(waits)):
                extra = self.nc.sync.nop()
                extra.ins.sync_info = mybir.SyncInfo(
                    on_wait=waits[i : i + 1], on_update=[]
                )
        self.nc.sync.drain()
        self.nc.all_engine_barrier()
        assert self.sems is not None
        popped = self.nc._tile_sem_poison_stack.pop()
        assert popped is self._sem_poison
        self.nc.clear_and_free_semaphores(list(self.sems.allocated().values()))
        self.nc.all_engine_barrier()

    tile_mod.TileContext._drain_and_barrier = _drain_and_barrier


# ----------------------------------------------------------------------------
# host-side sharding / stream building
# ----------------------------------------------------------------------------

def _host_prep(src, dst, n_nodes, n_cores):
    """Shard edges by src range, sort by src, build per-core device streams.

    Returns (cfg, per_core) where per_core[c] is a dict of numpy arrays and
    cfg holds the uniform shape parameters.
    """
    NV = n_nodes // n_cores
    NW = (NV + P - 1) // P
    src = np.asarray(src)
    dst = np.asarray(dst)

    cores = []
    for c in range(n_cores):
        lo = c * NV
        sel = (src >= lo) & (src < lo + NV)
        es = src[sel].astype(np.int64) - lo
        ed = dst[sel].astype(np.int64)
        order = np.argsort(es, kind="stable")
        es = es[order]
        ed = ed[order]
        u, counts = np.unique(es, return_counts=True)
        K = len(u)
        ranks = np.repeat(np.arange(K, dtype=np.int64), counts)
        w = ranks // P
        cnt_w = np.bincount(w, minlength=NW)
        cores.append((ed, u, K, ranks, w, cnt_w))

    T_w = 1
    for (_, _, _, _, _, cnt_w) in cores:
        T_w = max(T_w, int(math.ceil(cnt_w.max() / P)))

    per_core = []
    for c in range(n_cores):
        ed, u, K, ranks, w, cnt_w = cores[c]
        lo = c * NV
        nslot = T_w * P
        slot_rank = np.full((NW, nslot), -1.0, np.float32)
        slot_dst = np.zeros((NW, nslot), np.int32)
        offs = np.concatenate([[0], np.cumsum(cnt_w)])
        pos = np.arange(len(ed)) - offs[w]
        slot_rank[w, pos] = (ranks % P).astype(np.float32)
        slot_dst[w, pos] = ed.astype(np.int32)

        # device layouts: [128, NW*T_w] with slot (w, i, p) -> [p, w*T_w + i]
        rank_col = (
            slot_rank.reshape(NW, T_w, P).transpose(2, 0, 1).reshape(P, NW * T_w)
        )
        didx = slot_dst.reshape(NW, T_w, P).transpose(2, 0, 1).reshape(P, NW * T_w)

        # ST one-hot bytes: stb[n, (w, i, e)] == 1 iff rank of slot
        # (w, i, e) == n.   (e is the partition index of the edge.)
        stb = np.zeros((P, NW, T_w, P), np.uint8)
        sr = slot_rank.reshape(NW, T_w, P)
        wv, iv, evi = np.nonzero(sr >= 0)
        nv = sr[wv, iv, evi].astype(np.int64)
        stb[nv, wv, iv, evi] = 1
        stb = stb.reshape(P, NW * T_w * P)

        u_pad = np.zeros(NW * P, np.int32)
        u_pad[:K] = (u + lo).astype(np.int32)
        uidx = u_pad.reshape(NW, P).T.copy()  # [128, NW]

        per_core.append(
            dict(didx=didx, rankc=rank_col, stb=stb, uidx=uidx, u=u, K=K)
        )

    cfg = dict(NV=NV, NW=NW, T_w=T_w)
    return cfg, per_core


# ----------------------------------------------------------------------------
# device program
# ----------------------------------------------------------------------------

def _split_multi_waits(nc):
    """This env's walrus accepts at most ONE sync-wait command per
    instruction.  Move extra waits onto single-wait nops inserted just
    before the instruction on the same engine (same sequencer => identical
    semantics)."""
    import concourse.mybir as mybir

    cnt = 0
    for f in nc.m.functions:
        for blk in f.blocks:
            insts = blk.instructions
            out = []
            changed = False
            for ins in insts:
                si = ins.sync_info
                waits = list(si.on_wait) if si is not None else []
                if len(waits) > 1:
                    changed = True
                    for w in waits[:-1]:
                        cnt += 1
                        nop = mybir.InstNoOp(
                            name=f"wsplit_{cnt}", ins=[], outs=[]
                        )
                        nop.engine = ins.engine
                        nop.sync_info = mybir.SyncInfo(on_wait=[w], on_update=[])
                        out.append(nop)
                    si.on_wait = waits[-1:]
                    ins.sync_info = si
                out.append(ins)
            if changed:
                blk.instructions = out
    return cnt


def _build_nc(N, NW, T_w, TC=16, split_waits=True):
    _apply_tile_patch()
    import concourse.bass as bass
    import concourse.mybir as mybir
    import concourse.tile as tile
    from concourse.masks import make_identity

    f32 = mybir.dt.float32
    i32 = mybir.dt.int32
    u8 = mybir.dt.uint8
    AF = mybir.ActivationFunctionType
    OP = mybir.AluOpType
    IOOA = bass.IndirectOffsetOnAxis

    nc = bass.Bass()
    x_d = nc.declare_dram_parameter("x", [N, IN_F], f32, isOutput=False)
    wl_d = nc.declare_dram_parameter("wl", [IN_F, IN_F], f32, isOutput=False)
    aw_d = nc.declare_dram_parameter("aw", [L, 2 * IN_F], f32, isOutput=False)
    ew_d = nc.declare_dram_parameter("ew", [IN_F, D_EM], f32, isOutput=False)
    bl_d = nc.declare_dram_parameter("bl", [IN_F, 1], f32, isOutput=False)
    embb_d = nc.declare_dram_parameter("embb", [1, D_EM], f32, isOutput=False)
    attb_d = nc.declare_dram_parameter("attb", [1, L], f32, isOutput=False)
    didx_d = nc.declare_dram_parameter("didx", [P, NW * T_w], i32, isOutput=False)
    rankc_d = nc.declare_dram_parameter("rankc", [P, NW * T_w], f32, isOutput=False)
    stb_d = nc.declare_dram_parameter("stb", [P, NW * T_w * P], u8, isOutput=False)
    uidx_d = nc.declare_dram_parameter("uidx", [P, NW], i32, isOutput=False)
    iota_d = nc.declare_dram_parameter("iota_mat", [P, P], f32, isOutput=False)
    descale_d = nc.declare_dram_parameter("descale", [P, 1], f32, isOutput=False)
    out_d = nc.declare_dram_parameter("out", [NW * P, 4 * D_EM], f32, isOutput=True)

    tbl = nc.dram_tensor("tbl", [N, P], f32)  # [h_em(64)|s_dst+attb(4)|s_src(4)|0]

    ntileA = (N + P - 1) // P
    chunks = []
    k0 = 0
    while k0 < T_w:
        chunks.append((k0, min(TC, T_w - k0)))
        k0 += TC

    with ExitStack() as ctx:
        tc = ctx.enter_context(tile.TileContext(nc))
        const = ctx.enter_context(tc.tile_pool(name="const", bufs=1))

        ident = const.tile([P, P], f32)
        make_identity(nc, ident[:])
        iota = const.tile([P, P], f32)
        nc.sync.dma_start(out=iota[:], in_=iota_d[:])
        descale = const.tile([P, 1], f32)
        nc.sync.dma_start(out=descale[:], in_=descale_d[:])

        # ---- fold weights: Wp[ic] = [emb_w | W@a_dst.T | W@a_src.T | 0] ----
        WT = [[const.tile([P, P], f32, name=f"WT_{j}_{i}") for i in range(2)] for j in range(2)]
        adT = [const.tile([P, L], f32, name=f"adT_{j}") for j in range(2)]
        asT = [const.tile([P, L], f32, name=f"asT_{j}") for j in range(2)]
        blT = [const.tile([P, 1], f32, name=f"blT_{j}") for j in range(2)]
        Wp = [const.tile([P, P], f32, name=f"Wp_{i}") for i in range(2)]
        bias_row = const.tile([1, P], f32)
        ones1 = const.tile([1, P], f32)
        bias_bc = const.tile([P, P], f32)
        attb_sb = const.tile([1, L], f32)

        with (
            tc.tile_pool(name="setup_sb", bufs=2) as ssb,
            tc.tile_pool(name="setup_ps", bufs=2, space="PSUM") as sps,
        ):
            for jc in range(2):
                nc.sync.dma_start(
                    out=adT[jc][:],
                    in_=aw_d[:, IN_F + jc * P : IN_F + (jc + 1) * P].transpose([1, 0]),
                )
                nc.sync.dma_start(
                    out=asT[jc][:],
                    in_=aw_d[:, jc * P : (jc + 1) * P].transpose([1, 0]),
                )
                nc.sync.dma_start(out=blT[jc][:], in_=bl_d[jc * P : (jc + 1) * P, :])
                for ic in range(2):
                    wt = ssb.tile([P, P], f32)
                    nc.sync.dma_start(
                        out=wt[:],
                        in_=wl_d[ic * P : (ic + 1) * P, jc * P : (jc + 1) * P],
                    )
                    tp = sps.tile([P, P], f32, space="PSUM")
                    nc.tensor.transpose(out=tp[:], in_=wt[:], identity=ident[:])
                    nc.vector.tensor_copy(out=WT[jc][ic][:], in_=tp[:])

            for ic in range(2):
                nc.gpsimd.memset(Wp[ic][:], 0)
                nc.sync.dma_start(
                    out=Wp[ic][:, 0:D_EM], in_=ew_d[ic * P : (ic + 1) * P, :]
                )
                wd_ps = sps.tile([P, 2 * L], f32, space="PSUM")
                for t, rhs_t in ((0, adT), (1, asT)):
                    for jc in range(2):
                        nc.tensor.matmul(
                            out=wd_ps[:, t * L : (t + 1) * L],
                            lhsT=WT[jc][ic][:],
                            rhs=rhs_t[jc][:],
                            start=(jc == 0),
                            stop=(jc == 1),
                        )
                nc.vector.tensor_copy(
                    out=Wp[ic][:, D_EM : D_EM + 2 * L], in_=wd_ps[:]
                )

            bias_ps = sps.tile([1, 2 * L], f32, space="PSUM")
            for t, rhs_t in ((0, adT), (1, asT)):
                for jc in range(2):
                    nc.tensor.matmul(
                        out=bias_ps[:, t * L : (t + 1) * L],
                        lhsT=blT[jc][:],
                        rhs=rhs_t[jc][:],
                        start=(jc == 0),
                        stop=(jc == 1),
                    )
            nc.gpsimd.memset(bias_row[:], 0)
            nc.sync.dma_start(out=bias_row[:, 0:D_EM], in_=embb_d[:])
            nc.sync.dma_start(out=attb_sb[:], in_=attb_d[:])
            nc.vector.tensor_tensor(
                out=bias_row[:, D_EM : D_EM + L],
                in0=bias_ps[:, 0:L],
                in1=attb_sb[:],
                op=OP.add,
            )
            nc.vector.tensor_copy(
                out=bias_row[:, D_EM + L : D_EM + 2 * L], in_=bias_ps[:, L : 2 * L]
            )
            # broadcast bias_row across partitions via K=1 matmul
            nc.gpsimd.memset(ones1[:], 1.0)
            bb_ps = sps.tile([P, P], f32, space="PSUM")
            nc.tensor.matmul(
                out=bb_ps[:], lhsT=ones1[:], rhs=bias_row[:], start=True, stop=True
            )
            nc.vector.tensor_copy(out=bias_bc[:], in_=bb_ps[:])

        # ---- phase A: build tbl[N, 128] ----
        with (
            tc.tile_pool(name="xa", bufs=3) as xa,
            tc.tile_pool(name="xt", bufs=3) as xtp,
            tc.tile_pool(name="stg", bufs=3) as stg,
            tc.tile_pool(name="psT", bufs=2, space="PSUM") as psT,
            tc.tile_pool(name="psM", bufs=2, space="PSUM") as psM,
        ):
            for i in range(ntileA):
                r0 = i * P
                pp = min(P, N - r0)
                xt = xa.tile([P, IN_F], f32)
                nc.sync.dma_start(out=xt[:pp, :], in_=x_d[r0 : r0 + pp, :])
                xTs = []
                for jc in range(2):
                    tp = psT.tile([P, P], f32, space="PSUM")
                    nc.tensor.transpose(
                        out=tp[:, :pp],
                        in_=xt[:pp, jc * P : (jc + 1) * P],
                        identity=ident[:pp, :pp],
                    )
                    xT = xtp.tile([P, P], f32)
                    nc.scalar.copy(out=xT[:, :pp], in_=tp[:, :pp])
                    xTs.append(xT)
                tab_ps = psM.tile([P, P], f32, space="PSUM")
                for jc in range(2):
                    nc.tensor.matmul(
                        out=tab_ps[:pp, :],
                        lhsT=xTs[jc][:, :pp],
                        rhs=Wp[jc][:],
                        start=(jc == 0),
                        stop=(jc == 1),
                    )
                st = stg.tile([P, P], f32)
                nc.vector.tensor_tensor(
                    out=st[:pp, :], in0=tab_ps[:pp, :], in1=bias_bc[:pp, :], op=OP.add
                )
                nc.sync.dma_start(out=tbl[r0 : r0 + pp, :], in_=st[:pp, :])

        # ---- upfront: s_src gather + streams ----
        uix = const.tile([P, NW], i32)
        nc.sync.dma_start(out=uix[:], in_=uidx_d[:])
        ssrc = const.tile([P, NW, L], f32)
        for w in range(NW):
            nc.gpsimd.indirect_dma_start(
                out=ssrc[:, w, :],
                out_offset=None,
                in_=tbl[:, :],
                in_offset=IOOA(ap=uix[:, w : w + 1], axis=0),
                element_offset=D_EM + L,
            )
        didx_sb = const.tile([P, NW * T_w], i32)
        nc.sync.dma_start(out=didx_sb[:], in_=didx_d[:])
        rankc_sb = const.tile([P, NW * T_w], f32)
        nc.sync.dma_start(out=rankc_sb[:], in_=rankc_d[:])

        # ---- edge phase ----
        with (
            tc.tile_pool(name="g", bufs=3) as gpool,
            tc.tile_pool(name="stb", bufs=3) as stbp,
            tc.tile_pool(name="st", bufs=3) as stp,
            tc.tile_pool(name="s", bufs=3) as sp,
            tc.tile_pool(name="z", bufs=4) as zp,
            tc.tile_pool(name="rev", bufs=2) as revp,
            tc.tile_pool(name="onorm", bufs=2) as onp,
            tc.tile_pool(name="psZ", bufs=3, space="PSUM") as psZ,
            tc.tile_pool(name="psU", bufs=2, space="PSUM") as psU,
        ):
            for w in range(NW):
                U_ps = psU.tile([P, 4 * D_EM + L], f32, space="PSUM")
                for (k0, tcw) in chunks:
                    c0 = w * T_w + k0
                    G = gpool.tile([P, TC, P], f32)
                    for i in range(tcw):
                        nc.gpsimd.indirect_dma_start(
                            out=G[:, i, :],
                            out_offset=None,
                            in_=tbl[:, :],
                            in_offset=IOOA(
                                ap=didx_sb[:, c0 + i : c0 + i + 1], axis=0
                            ),
                        )
                    stbits = stbp.tile([P, TC, P], u8)
                    nc.sync.dma_start(
                        out=stbits[:, :tcw, :],
                        in_=stb_d[:, c0 * P : (c0 + tcw) * P],
                    )
                    ST = stp.tile([P, TC, P], f32)
                    nc.scalar.copy(out=ST[:, :tcw, :], in_=stbits[:, :tcw, :])
                    S = sp.tile([P, TC, P], f32)
                    nc.vector.tensor_tensor(
                        out=S[:, :tcw, :],
                        in0=rankc_sb[:, c0 : c0 + tcw]
                        .unsqueeze(2)
                        .to_broadcast([P, tcw, P]),
                        in1=iota[:].unsqueeze(1).to_broadcast([P, tcw, P]),
                        op=OP.is_equal,
                    )
                    se_ps = psZ.tile([P, TC, L], f32, space="PSUM")
                    for i in range(tcw):
                        nc.tensor.matmul(
                            out=se_ps[:, i, :],
                            lhsT=ST[:, i, :],
                            rhs=ssrc[:, w, :],
                            start=True,
                            stop=True,
                        )
                    zt = zp.tile([P, TC, L], f32)
                    nc.vector.tensor_tensor(
                        out=zt[:, :tcw, :],
                        in0=se_ps[:, :tcw, :],
                        in1=G[:, :tcw, D_EM : D_EM + L],
                        op=OP.add,
                    )
                    sg = zp.tile([P, TC, L], f32)
                    nc.scalar.activation(
                        out=sg[:, :tcw, :], in_=zt[:, :tcw, :], func=AF.Sigmoid
                    )
                    Rev = revp.tile([P, TC, 4 * D_EM + L], f32)
                    nc.scalar.activation(
                        out=Rev[:, :tcw, 4 * D_EM : 4 * D_EM + L],
                        in_=sg[:, :tcw, :],
                        func=AF.Exp,
                    )
                    nc.vector.tensor_tensor(
                        out=Rev[:, :tcw, 0 : 4 * D_EM].rearrange(
                            "p t (l d) -> p t l d", l=L
                        ),
                        in0=G[:, :tcw, 0:D_EM]
                        .unsqueeze(2)
                        .to_broadcast([P, tcw, L, D_EM]),
                        in1=Rev[:, :tcw, 4 * D_EM : 4 * D_EM + L]
                        .unsqueeze(3)
                        .to_broadcast([P, tcw, L, D_EM]),
                        op=OP.mult,
                    )
                    for i in range(tcw):
                        nc.tensor.matmul(
                            out=U_ps[:, :],
                            lhsT=S[:, i, :],
                            rhs=Rev[:, i, :],
                            start=(k0 == 0 and i == 0),
                            stop=(k0 + tcw == T_w and i == tcw - 1),
                        )
                dn = onp.tile([P, L], f32)
                nc.vector.tensor_scalar(
                    out=dn[:],
                    in0=U_ps[:, 4 * D_EM : 4 * D_EM + L],
                    scalar1=1e-30,
                    scalar2=None,
                    op0=OP.add,
                )
                dnr = onp.tile([P, L], f32)
                nc.vector.reciprocal(out=dnr[:], in_=dn[:])
                ot = onp.tile([P, 4 * D_EM], f32)
                nc.vector.tensor_tensor(
                    out=ot[:].rearrange("p (l d) -> p l d", l=L),
                    in0=U_ps[:, 0 : 4 * D_EM].rearrange("p (l d) -> p l d", l=L),
                    in1=dnr[:].unsqueeze(2).to_broadcast([P, L, D_EM]),
                    op=OP.mult,
                )
                nc.sync.dma_start(out=out_d[w * P : (w + 1) * P, :], in_=ot[:])

    if split_waits:
        _split_multi_waits(nc)
    return nc


# ----------------------------------------------------------------------------
# public entry point
# ----------------------------------------------------------------------------

_NC_CACHE = {}


def _get_nc(N, NW, T_w, TC=16):
    key = (N, NW, T_w, TC)
    if key not in _NC_CACHE:
        _NC_CACHE[key] = _build_nc(N, NW, T_w, TC)
    return _NC_CACHE[key]


def _make_in_maps(x, W_lin, b_lin, att_w, att_b, emb_w, emb_b, per_core, n_cores):
    x = np.ascontiguousarray(np.asarray(x, np.float32))
    shared = dict(
        x=x,
        wl=np.ascontiguousarray(np.asarray(W_lin, np.float32)),
        aw=np.ascontiguousarray(np.asarray(att_w, np.float32)),
        ew=np.ascontiguousarray(np.asarray(emb_w, np.float32)),
        bl=np.ascontiguousarray(np.asarray(b_lin, np.float32).reshape(-1, 1)),
        embb=np.ascontiguousarray(np.asarray(emb_b, np.float32).reshape(1, -1)),
        attb=np.ascontiguousarray(np.asarray(att_b, np.float32).reshape(1, -1)),
        iota_mat=np.broadcast_to(
            np.arange(P, dtype=np.float32), (P, P)
        ).copy(),
        descale=(1.0 / (1 << (np.arange(P) // 16))).astype(np.float32).reshape(P, 1),
    )
    in_maps = []
    for c in range(n_cores):
        m = dict(shared)
        m["didx"] = per_core[c]["didx"]
        m["rankc"] = per_core[c]["rankc"]
        m["stb"] = per_core[c]["stb"]
        m["uidx"] = per_core[c]["uidx"]
        in_maps.append(m)
    return in_maps


def kernel(x, src, dst, W_lin, b_lin, att_w, att_b, emb_w, emb_b):
    from concourse.bass_utils import run_bass_kernel_spmd

    x = np.asarray(x)
    N = x.shape[0]
    cfg, per_core = _host_prep(src, dst, N, CORES)
    nc = _get_nc(N, cfg["NW"], cfg["T_w"])
    in_maps = _make_in_maps(
        x, W_lin, b_lin, att_w, att_b, emb_w, emb_b, per_core, CORES
    )
    res = run_bass_kernel_spmd(nc, in_maps, list(range(CORES)))
    out = np.zeros((N, 4 * D_EM), np.float32)
    NV = cfg["NV"]
    for c in range(CORES):
        K = per_core[c]["K"]
        u = per_core[c]["u"]
        out[c * NV + u] = res.results[c]["out"][:K]
    return out



# revision 2
# speedup vs baseline: 1.9118x; 1.9118x over previous
"""Bass/Trainium2 kernel for nn_DisentangleLayer (FactorGCN-style GNN layer).

Math (per reference):
  h    = x @ W_lin + b_lin                    [N, 256]
  h_em = x @ emb_w + emb_b                    [N, 64]
  s_src = h @ a_src.T ; s_dst = h @ a_dst.T   [N, 4]    (att_w = [a_src | a_dst])
  e    = sigmoid(s_src[src] + s_dst[dst] + att_b)       [E, 4]
  ev   = exp(e)              (the reference's exp(e - max e) cancels in
                              ev/denom exactly, and e is bounded in (0,1))
  denom = segsum_src(ev)                      [N, 4]
  out[n, 64l:64l+64] = segsum_src(ev_l * h_em[dst]) / denom[n, l] + emb_b

Key algebraic folds (weights-only, done host-side):
  w_dst = W_lin @ a_dst.T          [256, 4]
  w_src = W_lin @ a_src.T          [256, 4]
  attb_eff = att_b + b_lin @ a_src.T + b_lin @ a_dst.T
  the emb_b bias commutes with the attn-weighted average (weights sum to 1
  after normalization), so it is added once after the normalize step.

Strategy ("streamed slots" — no device-side random access):
  * Edges are sharded by src range across 8 cores (each core owns 12500
    nodes' outputs; no cross-core reduction).
  * Per core, nodes are ranked by descending degree; rank r lives at
    (window w = r//128, partition p = r%128).  Edge j of rank r occupies
    slot (w, tile t=j, partition p) — the "diagonal" layout: a slot's
    partition IS its source rank, so per-window s_src broadcasts along
    the free axis for free and the per-window segment sum is a chain of
    identity matmuls accumulating in PSUM.  Degree sorting makes the
    per-window tile count T_w track the max degree tightly (~1.3% pad).
  * The host streams x[dst[slot]] as bf16 columns (pure input
    rearrangement — no host FLOPs on x).  The device computes, per slot,
    [h_em | s_dst] = xs_tile @ [emb_w | w_dst] with TensorE (this is
    where the model's matmul FLOPs run), then sigmoid/exp on ScalarE,
    the ev*h_em outer product on VectorE (2x mode via pair-duplicated
    ev), and the per-window segment sum via identity matmuls into PSUM
    ([4 heads x 64 | ev] = 260 f32 columns).  denom comes out of the
    same accumulation; normalize = scale by 1/denom + emb_b.
  * s_src per rank comes from a small separate pass over the core's own
    12544 nodes (x_own @ w_src + attb_eff).
"""

import math
import numpy as np
from contextlib import ExitStack

P = 128
CORES = 8
IN_F = 256
D_EM = 64
L = 4
HW = D_EM + L          # 68 cols: [h_em | s_dst]
UC = 4 * D_EM + L      # 260 cols: [4 heads x 64 | ev]
GT = 7                 # slot-matmul PSUM group (7*68*4B = 1904B < 2KB bank)

_PATCHED = False


def _apply_tile_patch():
    """walrus in this env rejects >1 sem-wait on one instruction; split the
    TileContext exit-drain waits across single-wait nops."""
    global _PATCHED
    if _PATCHED:
        return
    _PATCHED = True
    import concourse.tile as tile_mod
    import concourse.mybir as mybir
    from concourse.vector_clock import ScopedClock

    def _drain_and_barrier(self, tick_clock, wait_clock):
        nop = self.nc.sync.nop()
        wait_clock.add_sem_waits(nop.ins, ScopedClock({None: tick_clock.global_clock}))
        si = nop.ins.sync_info
        waits = list(si.on_wait) if si is not None else []
        if len(waits) > 1:
            si.on_wait = waits[:1]
            nop.ins.sync_info = si
            for i in range(1, len(waits)):
                extra = self.nc.sync.nop()
                extra.ins.sync_info = mybir.SyncInfo(
                    on_wait=waits[i : i + 1], on_update=[]
                )
        self.nc.sync.drain()
        self.nc.all_engine_barrier()
        assert self.sems is not None
        popped = self.nc._tile_sem_poison_stack.pop()
        assert popped is self._sem_poison
        self.nc.clear_and_free_semaphores(list(self.sems.allocated().values()))
        self.nc.all_engine_barrier()

    tile_mod.TileContext._drain_and_barrier = _drain_and_barrier


def _split_multi_waits(nc):
    """This env's walrus accepts at most ONE sync-wait command per
    instruction.  Move extra waits onto single-wait nops inserted just
    before the instruction on the same engine (same sequencer => identical
    semantics)."""
    import concourse.mybir as mybir

    cnt = 0
    for f in nc.m.functions:
        for blk in f.blocks:
            insts = blk.instructions
            out = []
            changed = False
            for ins in insts:
                si = ins.sync_info
                waits = list(si.on_wait) if si is not None else []
                if len(waits) > 1:
                    changed = True
                    for w in waits[:-1]:
                        cnt += 1
                        nop = mybir.InstNoOp(
                            name=f"wsplit_{cnt}", ins=[], outs=[]
                        )
                        nop.engine = ins.engine
                        nop.sync_info = mybir.SyncInfo(on_wait=[w], on_update=[])
                        out.append(nop)
                    si.on_wait = waits[-1:]
                    ins.sync_info = si
                out.append(ins)
            if changed:
                blk.instructions = out
    return cnt


# ----------------------------------------------------------------------------
# host-side sharding / stream building
# ----------------------------------------------------------------------------

def _host_prep(src, dst, n_nodes, n_cores):
    """Shard edges by src range; build the degree-sorted diagonal slot
    layout and a uniform cross-core tile schedule.

    Returns (cfg, per_core): cfg has the shared schedule; per_core[c] has
    the per-core slot->dst map, rank permutation and valid mask.
    """
    src = np.asarray(src)
    dst = np.asarray(dst)
    NV = n_nodes // n_cores
    NW = (NV + P - 1) // P
    NR = NW * P

    cores = []
    tw_req = np.zeros((n_cores, NW), np.int64)
    for c in range(n_cores):
        lo = c * NV
        sel = (src >= lo) & (src < lo + NV)
        es = src[sel].astype(np.int64) - lo
        ed = dst[sel].astype(np.int64)
        deg = np.bincount(es, minlength=NV)
        perm = np.argsort(-deg, kind="stable")      # rank -> local node id
        rankof = np.empty(NV, np.int64)
        rankof[perm] = np.arange(NV)
        degpad = np.zeros(NR, np.int64)
        degpad[:NV] = deg[perm]
        tw_req[c] = degpad.reshape(NW, P).max(axis=1)
        cores.append((es, ed, perm, rankof, degpad))

    Tw = np.maximum(np.maximum.reduce(tw_req), 1)   # uniform schedule
    CW = np.concatenate([[0], np.cumsum(Tw)])
    ST = int(CW[-1])

    per_core = []
    for c in range(n_cores):
        es, ed, perm, rankof, degpad = cores[c]
        r_e = rankof[es]
        order = np.argsort(r_e, kind="stable")
        r_s = r_e[order]
        ed_s = ed[order]
        cnt = degpad[: r_s.max() + 1] if len(r_s) else degpad
        starts = np.concatenate([[0], np.cumsum(np.bincount(r_s, minlength=NR))])
        j_s = np.arange(len(r_s)) - starts[r_s]
        w_s = r_s // P
        p_s = r_s % P
        col_s = CW[w_s] + j_s
        slot_dst = np.full((P, ST), -1, np.int64)
        slot_dst[p_s, col_s] = ed_s
        per_core.append(dict(slot_dst=slot_dst, perm=perm))

    cfg = dict(NV=NV, NW=NW, NR=NR, ST=ST, Tw=tuple(int(t) for t in Tw),
               CW=CW)
    return cfg, per_core


def _bf16(a):
    import ml_dtypes
    return np.asarray(a, np.float32).astype(ml_dtypes.bfloat16)


def _build_xcols(x_bf, ids, valid=None):
    """[n_ids] node-id list -> bf16 column stream [2, 128, n_ids] from
    x_bf [N, 256] (invalid ids -> zero columns)."""
    n = len(ids)
    out = np.zeros((n, IN_F), x_bf.dtype)
    if valid is None:
        valid = ids >= 0
    idc = np.where(valid, ids, 0)
    out[valid] = x_bf[idc[valid]]
    # [n, 256] -> [256, n] -> [2, 128, n]
    return np.ascontiguousarray(out.T.reshape(2, P, n))


def _make_in_maps(x, W_lin, b_lin, att_w, att_b, emb_w, emb_b, cfg, per_core,
                  n_cores):
    x = np.asarray(x, np.float32)
    W_lin = np.asarray(W_lin, np.float32)
    b_lin = np.asarray(b_lin, np.float32)
    att_w = np.asarray(att_w, np.float32)
    att_b = np.asarray(att_b, np.float32)
    emb_w = np.asarray(emb_w, np.float32)
    emb_b = np.asarray(emb_b, np.float32)

    a_src = att_w[:, :IN_F]
    a_dst = att_w[:, IN_F:]
    w_dst = W_lin @ a_dst.T                      # [256, 4]
    w_src = W_lin @ a_src.T                      # [256, 4]
    attb_eff = att_b + b_lin @ a_src.T + b_lin @ a_dst.T   # [4]

    x_bf = _bf16(x)
    wp = _bf16(np.concatenate([emb_w, w_dst], axis=1)).reshape(2, P, HW)
    ws = _bf16(w_src).reshape(2, P, L)
    ident = _bf16(np.eye(P, dtype=np.float32))
    attb_bc = np.broadcast_to(attb_eff.astype(np.float32), (P, L)).copy()
    embb_bc = np.broadcast_to(
        np.tile(emb_b, L).astype(np.float32), (P, 4 * D_EM)
    ).copy()

    NV, NR, ST = cfg["NV"], cfg["NR"], cfg["ST"]
    in_maps = []
    for c in range(n_cores):
        pc = per_core[c]
        slot_dst = pc["slot_dst"]                # [P, ST]
        ids_flat = slot_dst.T.reshape(-1)        # (col, p) order
        xs = _build_xcols(x_bf, ids_flat)        # [2, 128, ST*128]
        own_ids = np.full(NR, -1, np.int64)
        own_ids[:NV] = pc["perm"] + c * NV
        xo = _build_xcols(x_bf, own_ids)         # [2, 128, NR]
        valid = _bf16((slot_dst >= 0).astype(np.float32))   # [P, ST]
        in_maps.append(dict(
            xs=xs, xo=xo, valid=valid, wp=wp, ws=ws, ident=ident,
            attb=attb_bc, embb=embb_bc,
        ))
    return in_maps


# ----------------------------------------------------------------------------
# device program
# ----------------------------------------------------------------------------

def _build_nc(NR, ST, Tw):
    _apply_tile_patch()
    import concourse.bass as bass
    import concourse.mybir as mybir
    import concourse.tile as tile

    f32 = mybir.dt.float32
    bf16 = mybir.dt.bfloat16
    AF = mybir.ActivationFunctionType
    OP = mybir.AluOpType

    NW = NR // P
    assert len(Tw) == NW
    CW = [0]
    for t in Tw:
        CW.append(CW[-1] + t)
    TMAX = max(Tw)

    nc = bass.Bass()
    xs_d = nc.declare_dram_parameter("xs", [2, P, ST * P], bf16, isOutput=False)
    xo_d = nc.declare_dram_parameter("xo", [2, P, NR], bf16, isOutput=False)
    valid_d = nc.declare_dram_parameter("valid", [P, ST], bf16, isOutput=False)
    wp_d = nc.declare_dram_parameter("wp", [2, P, HW], bf16, isOutput=False)
    ws_d = nc.declare_dram_parameter("ws", [2, P, L], bf16, isOutput=False)
    ident_d = nc.declare_dram_parameter("ident", [P, P], bf16, isOutput=False)
    attb_d = nc.declare_dram_parameter("attb", [P, L], f32, isOutput=False)
    embb_d = nc.declare_dram_parameter("embb", [P, 4 * D_EM], f32, isOutput=False)
    out_d = nc.declare_dram_parameter("out", [NR, 4 * D_EM], f32, isOutput=True)

    with ExitStack() as ctx:
        tc = ctx.enter_context(tile.TileContext(nc))
        const = ctx.enter_context(tc.tile_pool(name="const", bufs=1))

        ident = const.tile([P, P], bf16)
        nc.sync.dma_start(out=ident[:], in_=ident_d[:])
        wp = [const.tile([P, HW], bf16, name=f"wp{k}") for k in range(2)]
        ws = [const.tile([P, L], bf16, name=f"ws{k}") for k in range(2)]
        for k in range(2):
            nc.sync.dma_start(out=wp[k][:], in_=wp_d[k, :, :])
            nc.sync.dma_start(out=ws[k][:], in_=ws_d[k, :, :])
        attb = const.tile([P, L], f32)
        nc.sync.dma_start(out=attb[:], in_=attb_d[:])
        embb = const.tile([P, 4 * D_EM], f32)
        nc.sync.dma_start(out=embb[:], in_=embb_d[:])
        validm = const.tile([P, ST], bf16)
        nc.sync.dma_start(out=validm[:], in_=valid_d[:])
        ssrc = const.tile([P, NW, L], bf16)

        # ---- pass 1: s_src per rank (the core's own nodes, rank order) ----
        with (
            tc.tile_pool(name="sxo", bufs=3) as sxo,
            tc.tile_pool(name="sps", bufs=3, space="PSUM") as sps,
        ):
            for rt in range(NW):
                xot = sxo.tile([P, 2, P], bf16)
                for k in range(2):
                    nc.sync.dma_start(
                        out=xot[:, k, :], in_=xo_d[k, :, rt * P : (rt + 1) * P]
                    )
                ps = sps.tile([P, L], f32, space="PSUM")
                for k in range(2):
                    nc.tensor.matmul(
                        out=ps[:], lhsT=xot[:, k, :], rhs=ws[k][:],
                        start=(k == 0), stop=(k == 1),
                    )
                nc.vector.tensor_tensor(
                    out=ssrc[:, rt, :], in0=ps[:], in1=attb[:], op=OP.add
                )

        # ---- edge phase: one window at a time ----
        with (
            tc.tile_pool(name="xs", bufs=2) as xsp,
            tc.tile_pool(name="hem", bufs=2) as hemp,
            tc.tile_pool(name="sc", bufs=2) as scp,
            tc.tile_pool(name="rev", bufs=2) as revp,
            tc.tile_pool(name="os", bufs=2) as osp,
            tc.tile_pool(name="hps", bufs=3, space="PSUM") as hps,
            tc.tile_pool(name="ups", bufs=2, space="PSUM") as ups,
        ):
            for w in range(NW):
                T = Tw[w]
                c0 = CW[w]
                xsw = xsp.tile([P, 2, TMAX, P], bf16)
                for k in range(2):
                    nc.sync.dma_start(
                        out=xsw[:, k, :T, :],
                        in_=xs_d[k, :, c0 * P : (c0 + T) * P],
                    )
                hem = hemp.tile([P, TMAX, HW], bf16)
                ngrp = (T + GT - 1) // GT
                for g in range(ngrp):
                    t0 = g * GT
                    tg = min(GT, T - t0)
                    hp = hps.tile([P, GT, HW], f32, space="PSUM")
                    for t in range(t0, t0 + tg):
                        for k in range(2):
                            nc.tensor.matmul(
                                out=hp[:, t - t0, :],
                                lhsT=xsw[:, k, t, :],
                                rhs=wp[k][:],
                                start=(k == 0), stop=(k == 1),
                            )
                    nc.scalar.copy(
                        out=hem[:, t0 : t0 + tg, :], in_=hp[:, :tg, :]
                    )

                # zt = s_dst + s_src  (broadcast along tiles), on gpsimd
                zt = scp.tile([P, TMAX, L], bf16)
                nc.gpsimd.tensor_tensor(
                    out=zt[:, :T, :],
                    in0=hem[:, :T, D_EM:HW],
                    in1=ssrc[:, w, :].unsqueeze(1).to_broadcast([P, T, L]),
                    op=OP.add,
                )
                sg = scp.tile([P, TMAX, L], bf16)
                nc.scalar.activation(
                    out=sg[:, :T, :], in_=zt[:, :T, :], func=AF.Sigmoid
                )
                ev = scp.tile([P, TMAX, L], bf16)
                nc.scalar.activation(
                    out=ev[:, :T, :], in_=sg[:, :T, :], func=AF.Exp
                )
                # pair-duplicated + masked ev: evm2[p,t,l,j] = ev*valid
                evm2 = scp.tile([P, TMAX, L, 2], bf16)
                nc.vector.tensor_tensor(
                    out=evm2[:, :T, :, :],
                    in0=ev[:, :T, :].unsqueeze(3).to_broadcast([P, T, L, 2]),
                    in1=validm[:, c0 : c0 + T]
                    .unsqueeze(2).unsqueeze(3).to_broadcast([P, T, L, 2]),
                    op=OP.mult,
                )
                # Rev[:, t, :] = [evm_l * hem_d (256) | evm (4)]
                rev = revp.tile([P, TMAX, UC], bf16)
                nc.vector.tensor_tensor(
                    out=rev[:, :T, 0 : 4 * D_EM].rearrange(
                        "p t (l a b) -> p t l a b", l=L, b=2
                    ),
                    in0=hem[:, :T, 0:D_EM].rearrange(
                        "p t (a b) -> p t a b", b=2
                    ).unsqueeze(2).to_broadcast([P, T, L, D_EM // 2, 2]),
                    in1=evm2[:, :T, :, :]
                    .unsqueeze(3).to_broadcast([P, T, L, D_EM // 2, 2]),
                    op=OP.mult,
                )
                nc.gpsimd.tensor_copy(
                    out=rev[:, :T, 4 * D_EM : UC],
                    in_=evm2[:, :T, :, 0],
                )
                # segment sum: PSUM += Rev_t via identity matmuls
                U = ups.tile([P, UC], f32, space="PSUM")
                for t in range(T):
                    nc.tensor.matmul(
                        out=U[:], lhsT=ident[:], rhs=rev[:, t, :],
                        start=(t == 0), stop=(t == T - 1),
                    )
                # normalize: out = U / denom + emb_b
                dn = osp.tile([P, L], f32)
                nc.vector.tensor_scalar(
                    out=dn[:], in0=U[:, 4 * D_EM : UC],
                    scalar1=1e-20, scalar2=None, op0=OP.add,
                )
                dnr = osp.tile([P, L], f32)
                nc.vector.reciprocal(out=dnr[:], in_=dn[:])
                osb = osp.tile([P, 4 * D_EM], f32)
                for l in range(L):
                    nc.scalar.activation(
                        out=osb[:, l * D_EM : (l + 1) * D_EM],
                        in_=U[:, l * D_EM : (l + 1) * D_EM],
                        func=AF.Copy,
                        scale=dnr[:, l : l + 1],
                    )
                ofin = osp.tile([P, 4 * D_EM], f32)
                nc.vector.tensor_tensor(
                    out=ofin[:], in0=osb[:], in1=embb[:], op=OP.add
                )
                nc.sync.dma_start(
                    out=out_d[w * P : (w + 1) * P, :], in_=ofin[:]
                )

    _split_multi_waits(nc)
    return nc


# ----------------------------------------------------------------------------
# public entry point
# ----------------------------------------------------------------------------

_NC_CACHE = {}


def _get_nc(cfg):
    key = (cfg["NR"], cfg["ST"], cfg["Tw"])
    if key not in _NC_CACHE:
        _NC_CACHE[key] = _build_nc(cfg["NR"], cfg["ST"], cfg["Tw"])
    return _NC_CACHE[key]


def kernel(x, src, dst, W_lin, b_lin, att_w, att_b, emb_w, emb_b):
    from concourse.bass_utils import run_bass_kernel_spmd

    x = np.asarray(x)
    N = x.shape[0]
    cfg, per_core = _host_prep(src, dst, N, CORES)
    nc = _get_nc(cfg)
    in_maps = _make_in_maps(
        x, W_lin, b_lin, att_w, att_b, emb_w, emb_b, cfg, per_core, CORES
    )
    res = run_bass_kernel_spmd(nc, in_maps, list(range(CORES)))
    out = np.zeros((N, 4 * D_EM), np.float32)
    NV = cfg["NV"]
    for c in range(CORES):
        perm = per_core[c]["perm"]
        out[c * NV + perm] = res.results[c]["out"][:NV]
    return out


# revision 12
# speedup vs baseline: 2.1317x; 1.1150x over previous
"""Bass/Trainium2 kernel for nn_DisentangleLayer (FactorGCN-style GNN layer).

Math (per reference):
  h    = x @ W_lin + b_lin                    [N, 256]
  h_em = x @ emb_w + emb_b                    [N, 64]
  s_src = h @ a_src.T ; s_dst = h @ a_dst.T   [N, 4]    (att_w = [a_src | a_dst])
  e    = sigmoid(s_src[src] + s_dst[dst] + att_b)       [E, 4]
  ev   = exp(e)              (the reference's exp(e - max e) cancels in
                              ev/denom exactly, and e is bounded in (0,1))
  denom = segsum_src(ev)                      [N, 4]
  out[n, 64l:64l+64] = segsum_src(ev_l * h_em[dst]) / denom[n, l] + emb_b

Key algebraic folds (weights-only, done host-side):
  w_dst = W_lin @ a_dst.T          [256, 4]
  w_src = W_lin @ a_src.T          [256, 4]
  attb_eff = att_b + b_lin @ a_src.T + b_lin @ a_dst.T
  the emb_b bias commutes with the attn-weighted average (weights sum to 1
  after normalization), so it is added once after the normalize step.

Strategy ("streamed slots" — no device-side random access):
  * Edges are sharded by src range across 8 cores (each core owns 12500
    nodes' outputs; no cross-core reduction).
  * Per core, nodes are ranked by descending degree; rank r lives at
    (window w = r//128, partition p = r%128).  Edge j of rank r occupies
    slot (w, tile t=j, partition p) — the "diagonal" layout: a slot's
    partition IS its source rank, so per-window s_src broadcasts along
    the free axis for free and the per-window segment sum is a chain of
    identity matmuls accumulating in PSUM.  Degree sorting makes the
    per-window tile count T_w track the max degree tightly (~1.3% pad).
  * The host streams x[dst[slot]] as bf16 columns (pure input
    rearrangement — no host FLOPs on x).  The device computes, per slot,
    [h_em | s_dst] = xs_tile @ [emb_w | w_dst] with TensorE (this is
    where the model's matmul FLOPs run), then sigmoid/exp on ScalarE,
    the ev*h_em outer product on VectorE (2x mode via pair-duplicated
    ev), and the per-window segment sum via identity matmuls into PSUM
    ([4 heads x 64 | ev] = 260 f32 columns).  denom comes out of the
    same accumulation; normalize = scale by 1/denom + emb_b.
  * s_src per rank comes from a small separate pass over the core's own
    12544 nodes (x_own @ w_src + attb_eff).
"""

import math
import numpy as np
from contextlib import ExitStack

P = 128
CORES = 8
IN_F = 256
D_EM = 64
L = 4
HW = D_EM + L          # 68 cols: [h_em | s_dst]
UC = 4 * D_EM + L      # 260 cols: [4 heads x 64 | ev]
GT = 7                 # slot-matmul PSUM group (7*68*4B = 1904B < 2KB bank)

_PATCHED = False


def _apply_tile_patch():
    """walrus in this env rejects >1 sem-wait on one instruction; split the
    TileContext exit-drain waits across single-wait nops."""
    global _PATCHED
    if _PATCHED:
        return
    _PATCHED = True
    import concourse.tile as tile_mod
    import concourse.mybir as mybir
    from concourse.vector_clock import ScopedClock

    def _drain_and_barrier(self, tick_clock, wait_clock):
        nop = self.nc.sync.nop()
        wait_clock.add_sem_waits(nop.ins, ScopedClock({None: tick_clock.global_clock}))
        si = nop.ins.sync_info
        waits = list(si.on_wait) if si is not None else []
        if len(waits) > 1:
            si.on_wait = waits[:1]
            nop.ins.sync_info = si
            for i in range(1, len(waits)):
                extra = self.nc.sync.nop()
                extra.ins.sync_info = mybir.SyncInfo(
                    on_wait=waits[i : i + 1], on_update=[]
                )
        self.nc.sync.drain()
        self.nc.all_engine_barrier()
        assert self.sems is not None
        popped = self.nc._tile_sem_poison_stack.pop()
        assert popped is self._sem_poison
        self.nc.clear_and_free_semaphores(list(self.sems.allocated().values()))
        self.nc.all_engine_barrier()

    tile_mod.TileContext._drain_and_barrier = _drain_and_barrier


def _split_multi_waits(nc):
    """This env's walrus accepts at most ONE sync-wait command per
    instruction.  Move extra waits onto single-wait nops inserted just
    before the instruction on the same engine (same sequencer => identical
    semantics)."""
    import concourse.mybir as mybir

    cnt = 0
    for f in nc.m.functions:
        for blk in f.blocks:
            insts = blk.instructions
            out = []
            changed = False
            for ins in insts:
                si = ins.sync_info
                waits = list(si.on_wait) if si is not None else []
                if len(waits) > 1:
                    changed = True
                    for w in waits[:-1]:
                        cnt += 1
                        nop = mybir.InstNoOp(
                            name=f"wsplit_{cnt}", ins=[], outs=[]
                        )
                        nop.engine = ins.engine
                        nop.sync_info = mybir.SyncInfo(on_wait=[w], on_update=[])
                        out.append(nop)
                    si.on_wait = waits[-1:]
                    ins.sync_info = si
                out.append(ins)
            if changed:
                blk.instructions = out
    return cnt


# ----------------------------------------------------------------------------
# host-side sharding / stream building
# ----------------------------------------------------------------------------

def _host_prep(src, dst, n_nodes, n_cores):
    """Shard edges by src range; build the degree-sorted diagonal slot
    layout and a uniform cross-core tile schedule.

    Returns (cfg, per_core): cfg has the shared schedule; per_core[c] has
    the per-core slot->dst map, rank permutation and valid mask.
    """
    src = np.asarray(src)
    dst = np.asarray(dst)
    NV = n_nodes // n_cores
    NW = (NV + P - 1) // P
    NR = NW * P

    cores = []
    tw_req = np.zeros((n_cores, NW), np.int64)
    for c in range(n_cores):
        lo = c * NV
        sel = (src >= lo) & (src < lo + NV)
        es = src[sel].astype(np.int64) - lo
        ed = dst[sel].astype(np.int64)
        deg = np.bincount(es, minlength=NV)
        perm = np.argsort(-deg, kind="stable")      # rank -> local node id
        rankof = np.empty(NV, np.int64)
        rankof[perm] = np.arange(NV)
        degpad = np.zeros(NR, np.int64)
        degpad[:NV] = deg[perm]
        tw_req[c] = degpad.reshape(NW, P).max(axis=1)
        cores.append((es, ed, perm, rankof, degpad))

    Tw = np.maximum(np.maximum.reduce(tw_req), 1)   # uniform schedule
    CW = np.concatenate([[0], np.cumsum(Tw)])
    ST = int(CW[-1])

    per_core = []
    for c in range(n_cores):
        es, ed, perm, rankof, degpad = cores[c]
        r_e = rankof[es]
        order = np.argsort(r_e, kind="stable")
        r_s = r_e[order]
        ed_s = ed[order]
        cnt = degpad[: r_s.max() + 1] if len(r_s) else degpad
        starts = np.concatenate([[0], np.cumsum(np.bincount(r_s, minlength=NR))])
        j_s = np.arange(len(r_s)) - starts[r_s]
        w_s = r_s // P
        p_s = r_s % P
        col_s = CW[w_s] + j_s
        slot_dst = np.full((P, ST), -1, np.int64)
        slot_dst[p_s, col_s] = ed_s
        per_core.append(dict(slot_dst=slot_dst, perm=perm))

    cfg = dict(NV=NV, NW=NW, NR=NR, ST=ST, Tw=tuple(int(t) for t in Tw),
               CW=CW)
    return cfg, per_core


def _bf16(a):
    import ml_dtypes
    return np.asarray(a, np.float32).astype(ml_dtypes.bfloat16)


def _build_xcols(x_bf, ids, valid=None):
    """[n_ids] node-id list -> bf16 column stream [2, 128, n_ids] from
    x_bf [N, 256] (invalid ids -> zero columns)."""
    n = len(ids)
    out = np.zeros((n, IN_F), x_bf.dtype)
    if valid is None:
        valid = ids >= 0
    idc = np.where(valid, ids, 0)
    out[valid] = x_bf[idc[valid]]
    # [n, 256] -> [256, n] -> [2, 128, n] -> [128, 2, n]  (p-major so a
    # window's two k-chunks load in ONE dma_start)
    return np.ascontiguousarray(out.T.reshape(2, P, n).transpose(1, 0, 2))


def _make_in_maps(x, W_lin, b_lin, att_w, att_b, emb_w, emb_b, cfg, per_core,
                  n_cores):
    x = np.asarray(x, np.float32)
    W_lin = np.asarray(W_lin, np.float32)
    b_lin = np.asarray(b_lin, np.float32)
    att_w = np.asarray(att_w, np.float32)
    att_b = np.asarray(att_b, np.float32)
    emb_w = np.asarray(emb_w, np.float32)
    emb_b = np.asarray(emb_b, np.float32)

    a_src = att_w[:, :IN_F]
    a_dst = att_w[:, IN_F:]
    w_dst = W_lin @ a_dst.T                      # [256, 4]
    w_src = W_lin @ a_src.T                      # [256, 4]
    attb_eff = att_b + b_lin @ a_src.T + b_lin @ a_dst.T   # [4]

    x_bf = _bf16(x)
    wp = _bf16(np.concatenate([emb_w, w_dst], axis=1)).reshape(2, P, HW)
    ws = _bf16(w_src).reshape(2, P, L)
    ident = _bf16(np.eye(P, dtype=np.float32))
    attb_bc = np.broadcast_to(attb_eff.astype(np.float32), (P, L)).copy()
    embb_bc = np.broadcast_to(
        np.tile(emb_b, L).astype(np.float32), (P, 4 * D_EM)
    ).copy()

    NV, NR, ST = cfg["NV"], cfg["NR"], cfg["ST"]
    in_maps = []
    for c in range(n_cores):
        pc = per_core[c]
        slot_dst = pc["slot_dst"]                # [P, ST]
        ids_flat = slot_dst.T.reshape(-1)        # (col, p) order
        xs = _build_xcols(x_bf, ids_flat)        # [2, 128, ST*128]
        own_ids = np.full(NR, -1, np.int64)
        own_ids[:NV] = pc["perm"] + c * NV
        xo = _build_xcols(x_bf, own_ids)         # [2, 128, NR]
        valid = _bf16((slot_dst >= 0).astype(np.float32))   # [P, ST]
        in_maps.append(dict(
            xs=xs, xo=xo, valid=valid, wp=wp, ws=ws, ident=ident,
            attb=attb_bc, embb=embb_bc,
        ))
    return in_maps


# ----------------------------------------------------------------------------
# device program
# ----------------------------------------------------------------------------

def _build_nc(NR, ST, Tw):
    _apply_tile_patch()
    import concourse.bass as bass
    import concourse.mybir as mybir
    import concourse.tile as tile

    f32 = mybir.dt.float32
    bf16 = mybir.dt.bfloat16
    AF = mybir.ActivationFunctionType
    OP = mybir.AluOpType

    NW = NR // P
    assert len(Tw) == NW
    CW = [0]
    for t in Tw:
        CW.append(CW[-1] + t)
    TMAX = max(Tw)

    nc = bass.Bass()
    xs_d = nc.declare_dram_parameter("xs", [P, 2, ST * P], bf16, isOutput=False)
    xo_d = nc.declare_dram_parameter("xo", [P, 2, NR], bf16, isOutput=False)
    valid_d = nc.declare_dram_parameter("valid", [P, ST], bf16, isOutput=False)
    wp_d = nc.declare_dram_parameter("wp", [2, P, HW], bf16, isOutput=False)
    ws_d = nc.declare_dram_parameter("ws", [2, P, L], bf16, isOutput=False)
    ident_d = nc.declare_dram_parameter("ident", [P, P], bf16, isOutput=False)
    attb_d = nc.declare_dram_parameter("attb", [P, L], f32, isOutput=False)
    embb_d = nc.declare_dram_parameter("embb", [P, 4 * D_EM], f32, isOutput=False)
    out_d = nc.declare_dram_parameter("out", [NR, 4 * D_EM], f32, isOutput=True)

    with ExitStack() as ctx:
        tc = ctx.enter_context(tile.TileContext(nc))
        const = ctx.enter_context(tc.tile_pool(name="const", bufs=1))

        ident = const.tile([P, P], bf16)
        nc.sync.dma_start(out=ident[:], in_=ident_d[:])
        wp = [const.tile([P, HW], bf16, name=f"wp{k}") for k in range(2)]
        ws = [const.tile([P, L], bf16, name=f"ws{k}") for k in range(2)]
        for k in range(2):
            nc.sync.dma_start(out=wp[k][:], in_=wp_d[k, :, :])
            nc.sync.dma_start(out=ws[k][:], in_=ws_d[k, :, :])
        attb = const.tile([P, L], f32)
        nc.sync.dma_start(out=attb[:], in_=attb_d[:])
        embb = const.tile([P, 4 * D_EM], f32)
        nc.sync.dma_start(out=embb[:], in_=embb_d[:])
        validm = const.tile([P, ST], bf16)
        nc.sync.dma_start(out=validm[:], in_=valid_d[:])
        ssrc = const.tile([P, NW, L], bf16)

        # ---- pass 1: s_src per rank (the core's own nodes, rank order) ----
        XOS = 8   # rank tiles per xo DMA strip
        with (
            tc.tile_pool(name="sxo", bufs=3) as sxo,
            tc.tile_pool(name="sps", bufs=3, space="PSUM") as sps,
        ):
            for s0 in range(0, NW, XOS):
                sn = min(XOS, NW - s0)
                xot = sxo.tile([P, 2, XOS, P], bf16)
                nc.sync.dma_start(
                    out=xot[:, :, :sn, :],
                    in_=xo_d[:, :, s0 * P : (s0 + sn) * P],
                )
                for j in range(sn):
                    ps = sps.tile([P, L], f32, space="PSUM")
                    for k in range(2):
                        nc.tensor.matmul(
                            out=ps[:], lhsT=xot[:, k, j, :], rhs=ws[k][:],
                            start=(k == 0), stop=(k == 1),
                        )
                    nc.vector.tensor_tensor(
                        out=ssrc[:, s0 + j, :], in0=ps[:], in1=attb[:],
                        op=OP.add,
                    )

        # ---- edge phase, software-pipelined across windows:
        #   iter w emits:  loads/slot-MMs/scores/Rev for w,
        #                  then segment-sum U for w-1 (PE FIFO: behind
        #                  hp(w), so PE never stalls on Rev(w-1)),
        #                  then normalize+store for w-1 (DVE FIFO: behind
        #                  Rev(w)).
        with (
            tc.tile_pool(name="xs", bufs=2) as xsp,
            tc.tile_pool(name="hem", bufs=2) as hemp,
            tc.tile_pool(name="sc", bufs=2) as scp,
            tc.tile_pool(name="rev", bufs=2) as revp,
            tc.tile_pool(name="os", bufs=2) as osp,
            tc.tile_pool(name="hps", bufs=3, space="PSUM") as hps,
            tc.tile_pool(name="ups", bufs=2, space="PSUM") as ups,
        ):
            def emit_u(st):
                # segment sum: PSUM += Rev_t via identity matmuls
                U = ups.tile([P, UC], f32, space="PSUM")
                T = st["T"]
                for t in range(T):
                    nc.tensor.matmul(
                        out=U[:], lhsT=ident[:], rhs=st["rev"][:, t, :],
                        start=(t == 0), stop=(t == T - 1),
                    )
                st["U"] = U

            def emit_tail(st):
                # normalize: out = U / denom + emb_b
                U = st["U"]
                w = st["w"]
                dn = osp.tile([P, L], f32)
                nc.vector.tensor_scalar(
                    out=dn[:], in0=U[:, 4 * D_EM : UC],
                    scalar1=1e-20, scalar2=None, op0=OP.add,
                )
                dnr = osp.tile([P, L], f32)
                nc.vector.reciprocal(out=dnr[:], in_=dn[:])
                osb = osp.tile([P, 4 * D_EM], f32)
                nc.vector.tensor_tensor(
                    out=osb[:].rearrange("p (l d) -> p l d", l=L),
                    in0=U[:, 0 : 4 * D_EM].rearrange("p (l d) -> p l d", l=L),
                    in1=dnr[:].unsqueeze(2).to_broadcast([P, L, D_EM]),
                    op=OP.mult,
                )
                ofin = osp.tile([P, 4 * D_EM], f32)
                nc.vector.tensor_tensor(
                    out=ofin[:], in0=osb[:], in1=embb[:], op=OP.add
                )
                nc.sync.dma_start(
                    out=out_d[w * P : (w + 1) * P, :], in_=ofin[:]
                )

            prev = None
            for w in range(NW):
                T = Tw[w]
                c0 = CW[w]
                xsw = xsp.tile([P, 2, TMAX, P], bf16)
                nc.sync.dma_start(
                    out=xsw[:, :, :T, :],
                    in_=xs_d[:, :, c0 * P : (c0 + T) * P],
                )
                hem = hemp.tile([P, TMAX, HW], bf16)
                ngrp = (T + GT - 1) // GT
                for g in range(ngrp):
                    t0 = g * GT
                    tg = min(GT, T - t0)
                    hp = hps.tile([P, GT, HW], f32, space="PSUM")
                    for t in range(t0, t0 + tg):
                        for k in range(2):
                            nc.tensor.matmul(
                                out=hp[:, t - t0, :],
                                lhsT=xsw[:, k, t, :],
                                rhs=wp[k][:],
                                start=(k == 0), stop=(k == 1),
                            )
                    nc.scalar.copy(
                        out=hem[:, t0 : t0 + tg, :], in_=hp[:, :tg, :]
                    )

                # zt = s_dst + s_src  (broadcast along tiles), on gpsimd
                zt = scp.tile([P, TMAX, L], bf16)
                nc.gpsimd.tensor_tensor(
                    out=zt[:, :T, :],
                    in0=hem[:, :T, D_EM:HW],
                    in1=ssrc[:, w, :].unsqueeze(1).to_broadcast([P, T, L]),
                    op=OP.add,
                )
                sg = scp.tile([P, TMAX, L], bf16)
                nc.scalar.activation(
                    out=sg[:, :T, :], in_=zt[:, :T, :], func=AF.Sigmoid
                )
                ev = scp.tile([P, TMAX, L], bf16)
                nc.scalar.activation(
                    out=ev[:, :T, :], in_=sg[:, :T, :], func=AF.Exp
                )
                # pair-duplicated + masked ev: evm2[p,t,l,j] = ev*valid
                evm2 = scp.tile([P, TMAX, L, 2], bf16)
                nc.vector.tensor_tensor(
                    out=evm2[:, :T, :, :],
                    in0=ev[:, :T, :].unsqueeze(3).to_broadcast([P, T, L, 2]),
                    in1=validm[:, c0 : c0 + T]
                    .unsqueeze(2).unsqueeze(3).to_broadcast([P, T, L, 2]),
                    op=OP.mult,
                )
                # Rev[:, t, :] = [evm_l * hem_d (256) | evm (4)]
                rev = revp.tile([P, TMAX, UC], bf16)
                nc.vector.tensor_tensor(
                    out=rev[:, :T, 0 : 4 * D_EM].rearrange(
                        "p t (l a b) -> p t l a b", l=L, b=2
                    ),
                    in0=hem[:, :T, 0:D_EM].rearrange(
                        "p t (a b) -> p t a b", b=2
                    ).unsqueeze(2).to_broadcast([P, T, L, D_EM // 2, 2]),
                    in1=evm2[:, :T, :, :]
                    .unsqueeze(3).to_broadcast([P, T, L, D_EM // 2, 2]),
                    op=OP.mult,
                )
                nc.gpsimd.tensor_copy(
                    out=rev[:, :T, 4 * D_EM : UC],
                    in_=evm2[:, :T, :, 0],
                )
                cur = dict(w=w, T=T, rev=rev)
                if prev is not None:
                    emit_u(prev)
                    emit_tail(prev)
                prev = cur
            emit_u(prev)
            emit_tail(prev)

    _split_multi_waits(nc)
    return nc


# ----------------------------------------------------------------------------
# public entry point
# ----------------------------------------------------------------------------

_NC_CACHE = {}


def _get_nc(cfg):
    key = (cfg["NR"], cfg["ST"], cfg["Tw"])
    if key not in _NC_CACHE:
        _NC_CACHE[key] = _build_nc(cfg["NR"], cfg["ST"], cfg["Tw"])
    return _NC_CACHE[key]


def kernel(x, src, dst, W_lin, b_lin, att_w, att_b, emb_w, emb_b):
    from concourse.bass_utils import run_bass_kernel_spmd

    x = np.asarray(x)
    N = x.shape[0]
    cfg, per_core = _host_prep(src, dst, N, CORES)
    nc = _get_nc(cfg)
    in_maps = _make_in_maps(
        x, W_lin, b_lin, att_w, att_b, emb_w, emb_b, cfg, per_core, CORES
    )
    res = run_bass_kernel_spmd(nc, in_maps, list(range(CORES)))
    out = np.zeros((N, 4 * D_EM), np.float32)
    NV = cfg["NV"]
    for c in range(CORES):
        perm = per_core[c]["perm"]
        out[c * NV + perm] = res.results[c]["out"][:NV]
    return out
